# revision 98
# baseline (speedup 1.0000x reference)
"""Bahdanau additive attention kernel for Trainium2 (8 NeuronCores, SPMD).

Problem: B=32, S=2048, ENC=DEC=ATT=1024 (fp32 inputs)
  u = enc @ U_a                [B,S,A]
  w = dec @ W_a                [B,1,A]
  e = tanh(w + u) @ v_t        [B,S,1]
  align = softmax(e, axis=1)
  context = align^T @ enc      [B,1,E]
  output = tanh([dec, context] @ ffn)   [B,1,D]
  returns (output, context)

Sharding: data-parallel over batch, 4 batches per core, weights replicated.

v2 design (vs v1): enc and U are DMA-cast fp32->fp8 directly (no bf16
copies, no DVE casts); the ctx matmul runs fp8 DoubleRow against the
same fp8 enc tiles, with softmax weights replicated 128-wide (PE
replicate-matmuls + DVE fp8 copies) and a second fp8 RESIDUAL pass
(rep8b) that cancels the weight-quantization error; dec/ctx transposes
for the ffn cat run on the PE (k-major relabeled to match big-descriptor
W/ffn loads) so the ACT engine streams pure tanh+exp; e-matmuls/exp of
unit k are emitted after unit k+1's u-block so PE never waits on the
tanh lag; the last batch's softmax/ctx work is front-run so only half
the chain trails the final tanh.

Per-core engine budget (TimelineSim, 151.4us total): ACT ~93us (128
tanh + 16 exp), PE ~95us (512 u-MM fp8 DR + e/rep/ctx x2/ffn/w), DMA
device ~73us (enc fp8 loads 23 + xbar transposes 29 + U/W/ffn 20).
Scheduler constraint that shaped the layout: DMAs issue through a
small in-flight window in the scheduler's global order with ~2us of
completion latency per hop, so every DMA's deps must be long-resolved
at its turn (transposes trail their input load by one pair slot, all
pair tiles are fresh buffers, stores go last).
"""

import numpy as np
import ml_dtypes

import concourse.bass as bass
import concourse.mybir as mybir
import concourse.tile as tile
from concourse import bacc
from concourse.bass_utils import run_bass_kernel_spmd

F32 = mybir.dt.float32
BF16 = mybir.dt.bfloat16
FP8 = mybir.dt.float8e4
U16 = mybir.dt.uint16
AF = mybir.ActivationFunctionType
DR = mybir.MatmulPerfMode.DoubleRow

U_SCALE = 1.0   # U loaded as raw fp8 (no scale)
V_SCALE = 32.0

B, S, E, A, D = 32, 2048, 1024, 1024, 1024
NCORES = 8
NB = B // NCORES          # 4 batches per core
P = 128
KE = E // P               # 8 e-chunks (128 each)
KB = 4                    # e-pair blocks (256 e-values each) for DoubleRow
MA = A // P               # 8 output chunks over att dim
KD = D // P               # 8 contraction chunks over dec dim
ST = S // P               # 16 s-tiles per batch
NQ = 4                    # units per batch
TQ = ST // NQ             # 4 s-tiles per unit (512 seq)
SQ = TQ * P               # 512 seq per unit
N512 = 512
NU = NB * NQ              # 16 units per core
NP = NU // 2              # 8 pair (2-unit) load/transpose groups


def _build_kernel_body(tc, repeat=1):
    nc = tc.nc
    enc = nc.dram_tensor("enc", [NB, S, E], F32, kind="ExternalInput")
    dec = nc.dram_tensor("dec", [NB, D], F32, kind="ExternalInput")
    U_a = nc.dram_tensor("U_a", [E, A], F32, kind="ExternalInput")
    W_a = nc.dram_tensor("W_a", [D, A], F32, kind="ExternalInput")
    v_t = nc.dram_tensor("v_t", [A, 1], F32, kind="ExternalInput")
    ffn = nc.dram_tensor("ffn", [D + E, D], F32, kind="ExternalInput")
    out = nc.dram_tensor("out", [NB, D], F32, kind="ExternalOutput")
    ctx_out = nc.dram_tensor("ctx_out", [NB, E], F32, kind="ExternalOutput")
    for _ in range(repeat):
        _build_once(tc, enc, dec, U_a, W_a, v_t, ffn, out, ctx_out)


def _build_once(tc, enc, dec, U_a, W_a, v_t, ffn, out, ctx_out):
    nc = tc.nc
    # s relabeled so each partition reads 4 CONSECUTIVE dram rows (one big
    # descriptor instead of 4): s = q*512 + p*4 + t. The relabeling flows
    # consistently through u/e/softmax/ctx (softmax is order-invariant and
    # every consumer uses the same tiling), so results are unchanged.
    enc_r = enc.rearrange("b (q p t) e -> b p q t e", q=NQ, p=P, t=TQ)
    # U rows paired (consecutive e) to match the fp8-in-u16 transpose:
    # U_sb[p, (kb c), a] = U[kb*256 + 2p + c, a]
    U_r = U_a.rearrange("(kb p c) a -> p kb c a", kb=KB, p=P, c=2)
    # W/ffn contractions relabeled k-major (d = p*8 + k) so each partition
    # reads consecutive dram rows -> 128-descriptor DMAs that don't choke
    # the SWDGE prep ring. The dec/ctx transposes below use matching
    # strided views, so results are unchanged.
    W_r = W_a.rearrange("(p k) a -> p k a", p=P)
    ffn_r = ffn.rearrange("(hf p c) d -> p hf c d", hf=2, p=P)

    with (
        tc.tile_pool(name="weights", bufs=1) as weights,
        tc.tile_pool(name="enc8", bufs=6) as enc8_pool,
        tc.tile_pool(name="encT8", bufs=4) as encT8_pool,
        tc.tile_pool(name="tanhp", bufs=9) as tanh_pool,
        tc.tile_pool(name="rows", bufs=1) as rows,
        tc.tile_pool(name="rows2", bufs=2) as rows2,
        tc.tile_pool(name="rows4", bufs=4) as rows4,
        tc.tile_pool(name="qtiles", bufs=1) as qtiles,
        tc.tile_pool(name="psum_u", bufs=3, space="PSUM") as psum_u,
        tc.tile_pool(name="psum_e", bufs=3, space="PSUM") as psum_e,
        tc.tile_pool(name="psum_c", bufs=1, space="PSUM") as psum_c,
    ):
        # ---------------- Pool-queue loads (device order matters) --------
        # Startup chain to the first tanh: enc(unit0) -> T(unit0) while
        # U8 (direct fp8) and W + wT land -> first tanh ~15us. Units 0/1
        # use separate tiles (dependency tracking is tile-granular).
        # dec rides the SP HWDGE queue as fp32 (no Pool prep, no cast) and
        # is transposed on the idle PE instead of the xbar; one flat [1, B*D]
        # tile (single descriptor, single DMA-window slot) keeps every
        # PE-transpose input at partition 0.
        dec_flat = rows.tile([1, NB * D], F32, tag="dec32")
        nc.sync.dma_start(out=dec_flat, in_=dec[:, :])

        # U loads straight to fp8 (gpsimd DMA cast, no scale): u already
        # tolerates ~3% fp8 noise on enc; the raw-range U quantization adds
        # ~1.2x to that one term and saves the bf16 load + DVE cast from
        # the startup critical path. Per-kb tiles keep deps precise.
        U8k = [weights.tile([P, 2, A], FP8, name=f"U8_{kb}")
               for kb in range(KB)]
        v_sb = weights.tile([P, MA], BF16)

        def load_U(kb):
            # full-a per-kb: partition p reads 2 consecutive dram rows (8KB)
            # per descriptor -> 128 descriptors/DMA (SWDGE-ring friendly)
            nc.gpsimd.dma_start(out=U8k[kb], in_=U_r[:, kb])

        def load_W_half(h):
            W_h = weights.tile([P, KD, N512], BF16, name=f"W_h{h}",
                               tag="Whalf")
            asl = slice(h * N512, (h + 1) * N512)
            for k in (0, 4):
                nc.gpsimd.dma_start(
                    out=W_h[:, k : k + 4, :], in_=W_r[:, k : k + 4, asl]
                )
            return W_h

        # unit_nat[k]: t -> (tile, local_t) natural fp8 enc for unit k
        unit_nat = {}

        def load_enc_pair(pp):
            nat = enc8_pool.tile([P, 2 * TQ, E], FP8, name=f"nat_{pp}",
                                 tag="enc8")
            b, q = divmod(2 * pp, NQ)
            nc.gpsimd.dma_start(out=nat, in_=enc_r[b, :, q : q + 2, :, :])
            unit_nat[2 * pp] = lambda t, nat=nat: (nat, t)
            unit_nat[2 * pp + 1] = lambda t, nat=nat: (nat, TQ + t)

        def load_enc_unit(k):
            nat = qtiles.tile([P, TQ, E], FP8, name=f"natu_{k}",
                              tag=f"enc8u{k}")
            b, q = divmod(k, NQ)
            nc.gpsimd.dma_start(out=nat, in_=enc_r[b, :, q, :, :])
            unit_nat[k] = lambda t, nat=nat: (nat, t)

        # ---------------- small shared tiles ----------------
        # catT[p, c, j] = cat[j, c*128+p] ; c 0..7 dec, 8..15 ctx (bf16,
        # written per-batch from PE transposes of ctx_row).
        catT = weights.tile([P, 2 * KE, NB], BF16)
        ones128 = weights.tile([1, P], BF16)
        nc.vector.memset(ones128, 1.0)
        id1 = weights.tile([1, 1], F32)
        nc.vector.memset(id1, 1.0)
        # dummy activation so the 1.3us LoadActFuncSet runs at t~0 instead
        # of right before the first real tanh
        act_warm = weights.tile([1, 1], F32)
        nc.scalar.activation(act_warm, id1, AF.Tanh)
        # dec transpose on PE, k-major to match W_r's row labeling:
        # catT[p, k, j] = dec[j, p*8+k]; per-row [1,128]->[128,1] transposes
        decT_ps = psum_u.tile([P, KE, NB], F32, tag="u")
        dec_kv = dec_flat.rearrange("o (j p2 k) -> o j k p2", j=NB, k=KD)
        for j in range(NB):
            for k in range(KD):
                nc.tensor.transpose(
                    decT_ps[:, k, j : j + 1], dec_kv[:, j, k, :], id1
                )
        nc.vector.tensor_copy(
            catT[:, 0:KE, :].rearrange("p c j -> p (c j)"),
            decT_ps.rearrange("p c j -> p (c j)"),
        )

        # W streams through ONE half-size tile (a-halves, sequentially):
        # the wT(m0-3) matmuls are emitted BETWEEN the two loads, so the
        # pool's WAR tracking orders the second load after them. Saves
        # 8KB/partition of SBUF (W is dead after startup), which pays for
        # the 4th encT8 transpose buffer.
        wT_ps = psum_c.tile([P, MA, NB], F32, tag="cvec")
        wT = weights.tile([P, MA, NB], F32)

        def emit_w_half(h, W_h):
            for m in range(4 * h, 4 * h + 4):
                for k in range(KD):
                    nc.tensor.matmul(
                        wT_ps[:, m, :],
                        lhsT=W_h[:, k, (m - 4 * h) * P : (m - 4 * h + 1) * P],
                        rhs=catT[:, k, :],
                        start=(k == 0),
                        stop=(k == KD - 1),
                    )
            sl = slice(4 * h, 4 * h + 4)
            nc.vector.tensor_copy(
                wT[:, sl, :].rearrange("p m b -> p (m b)"),
                wT_ps[:, sl, :].rearrange("p m b -> p (m b)"),
            )

        load_enc_unit(0)
        for kb in range(KB):
            load_U(kb)
        Wh = load_W_half(0)
        emit_w_half(0, Wh)
        load_enc_unit(1)
        Wh2 = load_W_half(1)
        emit_w_half(1, Wh2)
        nc.gpsimd.dma_start(
            out=v_sb, in_=v_t.rearrange("(m p) one -> p (m one)", p=P)
        )
        load_enc_pair(1)
        # remaining enc pairs + ffn are emitted inside the main loop.
        # NOTE on DMA ordering: the scheduler issues DMAs through a bounded
        # in-flight window in program order, so every DMA/transpose must
        # have its dependencies long-resolved by the time its turn comes;
        # transposes are emitted one pair-slot behind their input load, and
        # all pair tiles are fresh buffers (no WAR waits in the stream).

        # v8[p, mm, j, cc] = v[(2mm+j)*128+p] * 32 fp8, replicated 128 wide
        # (dual-fp8 Ldweights rejects narrow stationaries); the e-matmul
        # output is 128 identical rows of which exp reads row 0.
        v32 = weights.tile([P, MA], F32)
        nc.vector.tensor_scalar_mul(v32, v_sb, V_SCALE)
        zero128 = weights.tile([P, P], F32)
        nc.vector.memset(zero128, 0.0)
        v8 = weights.tile([P, MA // 2, 2, P], FP8)
        for mm in range(MA // 2):
            for j in range(2):
                nc.vector.tensor_scalar_add(
                    v8[:, mm, j, :], zero128, v32[:, 2 * mm + j : 2 * mm + j + 1]
                )

        # ffn_sb[p, hf, c, d] = ffn[hf*1024 + p*8 + c, d] (k-major halves:
        # hf=0 dec rows, hf=1 ctx rows); 4 DMAs of 64 big descriptors
        ffn_sb = weights.tile([P, 2, KD, D], BF16)

        def load_ffn():
            for hf in range(2):
                for c in (0, 4):
                    nc.gpsimd.dma_start(
                        out=ffn_sb[:, hf, c : c + 4, :],
                        in_=ffn_r[:, hf, c : c + 4, :],
                    )

        # ---------------- transposes (SP HWDGE queue) ----------------
        # eTp[p, (qq t k), (j c)]: fp8 pairs viewed as u16 through the xbar.
        # rhs_view[k] is a list of (view, t0, tn) segments; view dims are
        # [p, kb, c, t, j] fp8 slices of the transposed result.
        rhs_view = {}

        def emit_T_pair(pp):
            eTp = encT8_pool.tile([P, 8 * TQ, P], U16, name=f"eT_{pp}",
                                  tag="encT8")
            src = unit_nat[2 * pp](0)[0].rearrange(
                "p t e -> p (t e)").bitcast(U16)
            nc.sync.dma_start(out=eTp, in_=src, transpose=True)
            pair_view = eTp[:, :, :].bitcast(FP8).rearrange(
                "p (qq t k) (j c) -> p qq k c t j", qq=2, t=TQ, k=KB, c=2
            )
            rhs_view[2 * pp] = [(pair_view[:, 0], 0, TQ)]
            rhs_view[2 * pp + 1] = [(pair_view[:, 1], 0, TQ)]

        def emit_T_unit(k):
            eTu = qtiles.tile([P, 4 * TQ, P], U16, name=f"eTu_{k}",
                              tag=f"encT8u{k}")
            src = unit_nat[k](0)[0].rearrange("p t e -> p (t e)").bitcast(U16)
            nc.sync.dma_start(out=eTu, in_=src, transpose=True)
            rhs_view[k] = [(eTu[:, :, :].bitcast(FP8).rearrange(
                "p (t k) (j c) -> p k c t j", t=TQ, k=KB, c=2
            ), 0, TQ)]

        def emit_T_half_unit(k, h, nat):
            eTq = qtiles.tile([P, 2 * TQ, P], U16, name=f"eTq_{k}_{h}",
                              tag=f"encT8q{h}")
            src = nat.rearrange("p t e -> p (t e)").bitcast(U16)
            nc.sync.dma_start(out=eTq, in_=src, transpose=True)
            view = eTq[:, :, :].bitcast(FP8).rearrange(
                "p (t k) (j c) -> p k c t j", t=2, k=KB, c=2
            )
            rhs_view.setdefault(k, []).append((view, 2 * h, 2))

        emit_T_unit(0)
        emit_T_unit(1)

        # ---------------- per-unit / per-batch helpers ----------------
        bst = {}

        def batch_state(b):
            if b not in bst:
                bst[b] = {
                    "expe": rows2.tile([1, S], BF16, name=f"expe_{b}",
                                       tag="expe"),
                    "esum4": rows2.tile([1, NQ], F32, name=f"esum4_{b}",
                                        tag="esum4"),
                    "th2s": {},
                    "e_ps": {},
                }
            return bst[b]

        def build_u_block(k):
            """u matmuls (fp8 DR) + fused tanh (fp8 out) for unit k."""
            b, q = divmod(k, NQ)
            st = batch_state(b)
            th2s = []
            th2 = None
            for m in range(MA):
                u_ps = psum_u.tile([P, SQ], F32, name="u_ps", tag="u")
                for (seg, t0, tn) in rhs_view[k]:
                    for kb in range(KB):
                        nc.tensor.matmul(
                            u_ps[:, t0 * P : (t0 + tn) * P],
                            lhsT=U8k[kb][:, :, m * P : (m + 1) * P],
                            rhs=seg[:, kb],
                            start=(kb == 0),
                            stop=(kb == KB - 1),
                            perf_mode=DR,
                        )
                if m % 2 == 0:
                    th2 = tanh_pool.tile([P, 2, SQ], FP8, name="th2", tag="th")
                nc.scalar.activation(
                    th2[:, m % 2, :], u_ps, AF.Tanh,
                    bias=wT[:, m, b : b + 1],
                    scale=1.0 / U_SCALE,
                )
                if m % 2 == 1:
                    th2s.append(th2)
            st["th2s"][q] = th2s

        def emit_e_exp(k):
            """e-matmuls (fp8 DR over tanh pairs) + exp for unit k; emitted
            one unit late so PE/ACT never wait on the tanh lag."""
            b, q = divmod(k, NQ)
            st = batch_state(b)
            e_ps = psum_e.tile([P, SQ], F32, name=f"e_ps_{k}", tag="eps")
            for mm, t2 in enumerate(st["th2s"].pop(q)):
                nc.tensor.matmul(
                    e_ps,
                    lhsT=v8[:, mm, :, :],
                    rhs=t2,
                    start=(mm == 0),
                    stop=(mm == MA // 2 - 1),
                    perf_mode=DR,
                )
            # e_ps holds 32*e (v8 scaling), undone by the exp scale
            nc.scalar.activation(
                st["expe"][:, q * SQ : (q + 1) * SQ],
                e_ps[0:1, :],
                AF.Exp,
                scale=1.0 / V_SCALE,
                accum_out=st["esum4"][:, q : q + 1],
            )

        def emit_rep(b):
            """PE replicate-matmuls: expe8_rep[p, tg, :] = expe[tg*128+p]
            (fp8, 128-wide) in 4 single-bank chunks, DVE-copied to sbuf.
            rep8b holds the fp8 cast RESIDUAL (expe - fp8(expe)): the ctx
            matmul accumulates both, wiping the softmax-weight quantization
            error (~30% of ctx's error budget) for one extra DVE op and 16
            extra 107ns DR matmuls per batch."""
            alloc_rep(b)
            emit_rep_chunks(b, range(4))
            emit_sums(b)

        def alloc_rep(b):
            st = batch_state(b)
            st["rep8"] = rows2.tile([P, ST, P], FP8, name=f"rep8_{b}",
                                    tag="rep8")
            st["rep8b"] = rows2.tile([P, ST, P], FP8, name=f"rep8b_{b}",
                                     tag="rep8b")

        def emit_rep_chunks(b, chunks):
            st = batch_state(b)
            rep8, rep8b = st["rep8"], st["rep8b"]
            for c in chunks:
                rep_ps = psum_e.tile([P, 4 * P], F32, name=f"rep_ps_{b}_{c}",
                                     tag="eps")
                for t in range(4):
                    tg = c * 4 + t
                    nc.tensor.matmul(
                        rep_ps[:, t * P : (t + 1) * P],
                        lhsT=st["expe"][:, tg * P : (tg + 1) * P],
                        rhs=ones128,
                        start=True,
                        stop=True,
                    )
                sl = rep8[:, 4 * c : 4 * (c + 1), :].rearrange(
                    "p t j -> p (t j)")
                nc.vector.tensor_copy(sl, rep_ps)
                nc.vector.tensor_tensor(
                    rep8b[:, 4 * c : 4 * (c + 1), :].rearrange(
                        "p t j -> p (t j)"),
                    rep_ps, sl, mybir.AluOpType.subtract,
                )

        def emit_sums(b):
            st = batch_state(b)
            esum = rows2.tile([1, 1], F32, name=f"esumt_{b}", tag="esumt")
            nc.vector.tensor_reduce(esum, st["esum4"], mybir.AxisListType.X,
                                    mybir.AluOpType.add)
            rsum = rows2.tile([1, 1], F32, name=f"rsum_{b}", tag="rsum")
            nc.vector.reciprocal(rsum, esum)
            st["rsum"] = rsum

        def emit_ctx(b):
            """fp8 DoubleRow ctx matmuls + scale/copy-out (DVE)."""
            emit_ctx_mms(b, range(ST // 2))
            emit_ctx_fin(b)

        def emit_ctx_mms(b, urange):
            st = batch_state(b)
            if "ctx_ps" not in st:
                st["ctx_ps"] = psum_c.tile([P, 2, N512], F32,
                                           name=f"ctx_ps_{b}", tag="cvec")
            ctx_ps = st["ctx_ps"]
            for n in range(2):
                for h, rep in enumerate((st["rep8"], st["rep8b"])):
                    for u in urange:
                        q, t = divmod(2 * u, NQ)
                        nat, lt = unit_nat[b * NQ + q](t)
                        nc.tensor.matmul(
                            ctx_ps[:, n, :],
                            lhsT=rep[:, 2 * u : 2 * u + 2, :],
                            rhs=nat[:, lt : lt + 2,
                                    n * N512 : (n + 1) * N512],
                            start=(h == 0 and u == 0),
                            stop=(h == 1 and u == ST // 2 - 1),
                            perf_mode=DR,
                            skip_group_check=True,
                        )

        def emit_ctx_fin(b):
            st = batch_state(b)
            ctx_ps = st["ctx_ps"]
            ctx_row = rows4.tile([1, E], F32, name=f"ctx_row_{b}",
                                 tag="ctx_row")
            st["ctx_row"] = ctx_row
            nc.vector.tensor_scalar_mul(
                ctx_row, ctx_ps[0:1, :, :].rearrange("o n f -> o (n f)"),
                st["rsum"],
            )
            # ctx column of catT via PE transposes (keeps the tail off the
            # xbar and the SP queue)
            # k-major strided views to match ffn_r's ctx-half row labeling
            ctx_kv = ctx_row.rearrange("o (p2 c) -> o c p2", c=KE)
            ctxT_ps = psum_e.tile([P, KE], F32, name=f"ctxT_{b}", tag="eps")
            for c in range(KE):
                nc.tensor.transpose(
                    ctxT_ps[:, c : c + 1], ctx_kv[:, c, :], id1
                )
            nc.vector.tensor_copy(
                catT[:, KE : 2 * KE, b : b + 1].rearrange("p c j -> p (c j)"),
                ctxT_ps,
            )

        # ---------------- main pipeline ----------------
        # Unit k's u-block is emitted at slot k; e/exp of k-1 after it;
        # rep(b-1) after slot 4b+1's u-block; ctx(b-1) after slot 4b+2's
        # (so the DVE rep8 copies overlap u(4b+2)'s execution).
        for k in range(NU):
            if k % 2 == 0 and k // 2 + 2 < NP:
                load_enc_pair(k // 2 + 2)
            if k % 2 == 0 and 1 <= k // 2 + 1 < NP:
                emit_T_pair(k // 2 + 1)
            if k == 12:
                load_ffn()
            build_u_block(k)
            if k >= 1:
                emit_e_exp(k - 1)
            if k >= 5 and k % NQ == 1:
                emit_rep(k // NQ - 1)
            if k >= 6 and k % NQ == 2:
                emit_ctx(k // NQ - 1)
            # last batch: front-run softmax replication + the q0/q1 half of
            # the ctx matmuls so only ~half the chain trails the last tanh
            if k == NU - 2:
                alloc_rep(NB - 1)
                emit_rep_chunks(NB - 1, (0, 1))
            if k == NU - 1:
                emit_rep_chunks(NB - 1, (2,))
                emit_ctx_mms(NB - 1, (0, 1, 2, 3))
        emit_e_exp(NU - 1)
        emit_rep_chunks(NB - 1, (3,))
        emit_sums(NB - 1)
        emit_ctx_mms(NB - 1, (4, 5, 6, 7))
        emit_ctx_fin(NB - 1)

        # ---------------- final ffn (all batches at once) ----------------
        out_ps = psum_c.tile([NB, D], F32, tag="cvec")
        for hf in range(2):
            for c in range(KD):
                for n in range(2):
                    nc.tensor.matmul(
                        out_ps[:, n * N512 : (n + 1) * N512],
                        lhsT=catT[:, hf * KE + c, :],
                        rhs=ffn_sb[:, hf, c, n * N512 : (n + 1) * N512],
                        start=(hf == 0 and c == 0),
                        stop=(hf == 1 and c == KD - 1),
                    )
        out_sb = weights.tile([NB, D], F32)
        nc.scalar.activation(out_sb, out_ps, AF.Tanh)
        nc.scalar.dma_start(out=out[:, :], in_=out_sb)
        # ctx stores last: tiny DMAs whose deps resolved long ago, so they
        # never head-block the issue window mid-stream
        for b in range(NB):
            nc.scalar.dma_start(out=ctx_out[b : b + 1, :],
                                in_=bst[b]["ctx_row"])


_NC_CACHE = None


def _get_nc(repeat=1):
    global _NC_CACHE
    if repeat != 1:
        nc = bacc.Bacc(None, target_bir_lowering=False)
        with tile.TileContext(nc) as tc:
            _build_kernel_body(tc, repeat=repeat)
        nc.compile()
        return nc
    if _NC_CACHE is None:
        nc = bacc.Bacc(None, target_bir_lowering=False)
        with tile.TileContext(nc) as tc:
            _build_kernel_body(tc)
        nc.compile()
        _NC_CACHE = nc
    return _NC_CACHE


def kernel(encoder_hidden_states, decoder_hidden_state, U_a, W_a, v_t, ffn,
           _trace=False):
    enc = np.ascontiguousarray(np.asarray(encoder_hidden_states, dtype=np.float32))
    dec = np.ascontiguousarray(
        np.asarray(decoder_hidden_state, dtype=np.float32).reshape(B, D)
    )
    U = np.ascontiguousarray(np.asarray(U_a, dtype=np.float32))
    W = np.ascontiguousarray(np.asarray(W_a, dtype=np.float32))
    v = np.ascontiguousarray(np.asarray(v_t, dtype=np.float32))
    F = np.ascontiguousarray(np.asarray(ffn, dtype=np.float32))

    nc = _get_nc()
    in_maps = []
    for c in range(NCORES):
        sl = slice(c * NB, (c + 1) * NB)
        in_maps.append(
            {
                "enc": enc[sl],
                "dec": dec[sl],
                "U_a": U,
                "W_a": W,
                "v_t": v,
                "ffn": F,
            }
        )
    res = run_bass_kernel_spmd(nc, in_maps, core_ids=list(range(NCORES)),
                               trace=_trace)

    output = np.empty((B, 1, D), dtype=np.float32)
    context = np.empty((B, 1, E), dtype=np.float32)
    for c in range(NCORES):
        sl = slice(c * NB, (c + 1) * NB)
        output[sl, 0, :] = res.results[c]["out"]
        context[sl, 0, :] = res.results[c]["ctx_out"]
    if _trace:
        return (output, context), res
    return (output, context)


if __name__ == "__main__":
    import reference

    inputs = {k: np.asarray(v) for k, v in reference.setup_inputs().items()}
    (o, c) = kernel(**inputs)
    print("output", o.shape, o.dtype, "context", c.shape, c.dtype)


# revision 100
# speedup vs baseline: 1.0130x; 1.0130x over previous
"""Bahdanau additive attention kernel for Trainium2 (8 NeuronCores, SPMD).

Problem: B=32, S=2048, ENC=DEC=ATT=1024 (fp32 inputs)
  u = enc @ U_a                [B,S,A]
  w = dec @ W_a                [B,1,A]
  e = tanh(w + u) @ v_t        [B,S,1]
  align = softmax(e, axis=1)
  context = align^T @ enc      [B,1,E]
  output = tanh([dec, context] @ ffn)   [B,1,D]
  returns (output, context)

Sharding: data-parallel over batch, 4 batches per core, weights replicated.

v2 design (vs v1): enc and U are DMA-cast fp32->fp8 directly (no bf16
copies, no DVE casts); the ctx matmul runs fp8 DoubleRow against the
same fp8 enc tiles, with softmax weights replicated 128-wide (PE
replicate-matmuls + DVE fp8 copies) and a second fp8 RESIDUAL pass
(rep8b) that cancels the weight-quantization error; dec/ctx transposes
for the ffn cat run on the PE (k-major relabeled to match big-descriptor
W/ffn loads) so the ACT engine streams pure tanh+exp; e-matmuls/exp of
unit k are emitted after unit k+1's u-block so PE never waits on the
tanh lag; the last batch's softmax/ctx work is front-run so only half
the chain trails the final tanh.

Per-core engine budget (TimelineSim, 150.3us total): ACT ~93us (128
tanh + 16 exp), PE ~95us (512 u-MM fp8 DR + e/rep/ctx x2/ffn/w), DMA
device ~73us (enc fp8 loads 23 + xbar transposes 29 + U/W/ffn 20).
Scheduler constraint that shaped the layout: DMAs issue through a
small in-flight window in the scheduler's global order with ~2us of
completion latency per hop, so every DMA's deps must be long-resolved
at its turn (transposes trail their input load by one pair slot, all
pair tiles are fresh buffers, stores go last).
"""

import numpy as np
import ml_dtypes

import concourse.bass as bass
import concourse.mybir as mybir
import concourse.tile as tile
from concourse import bacc
from concourse.bass_utils import run_bass_kernel_spmd

F32 = mybir.dt.float32
BF16 = mybir.dt.bfloat16
FP8 = mybir.dt.float8e4
U16 = mybir.dt.uint16
AF = mybir.ActivationFunctionType
DR = mybir.MatmulPerfMode.DoubleRow

U_SCALE = 1.0   # U loaded as raw fp8 (no scale)
V_SCALE = 32.0

B, S, E, A, D = 32, 2048, 1024, 1024, 1024
NCORES = 8
NB = B // NCORES          # 4 batches per core
P = 128
KE = E // P               # 8 e-chunks (128 each)
KB = 4                    # e-pair blocks (256 e-values each) for DoubleRow
MA = A // P               # 8 output chunks over att dim
KD = D // P               # 8 contraction chunks over dec dim
ST = S // P               # 16 s-tiles per batch
NQ = 4                    # units per batch
TQ = ST // NQ             # 4 s-tiles per unit (512 seq)
SQ = TQ * P               # 512 seq per unit
N512 = 512
NU = NB * NQ              # 16 units per core
NP = NU // 2              # 8 pair (2-unit) load/transpose groups


def _build_kernel_body(tc, repeat=1):
    nc = tc.nc
    enc = nc.dram_tensor("enc", [NB, S, E], F32, kind="ExternalInput")
    dec = nc.dram_tensor("dec", [NB, D], F32, kind="ExternalInput")
    U_a = nc.dram_tensor("U_a", [E, A], F32, kind="ExternalInput")
    W_a = nc.dram_tensor("W_a", [D, A], F32, kind="ExternalInput")
    v_t = nc.dram_tensor("v_t", [A, 1], F32, kind="ExternalInput")
    ffn = nc.dram_tensor("ffn", [D + E, D], F32, kind="ExternalInput")
    out = nc.dram_tensor("out", [NB, D], F32, kind="ExternalOutput")
    ctx_out = nc.dram_tensor("ctx_out", [NB, E], F32, kind="ExternalOutput")
    for _ in range(repeat):
        _build_once(tc, enc, dec, U_a, W_a, v_t, ffn, out, ctx_out)


def _build_once(tc, enc, dec, U_a, W_a, v_t, ffn, out, ctx_out):
    nc = tc.nc
    # s relabeled so each partition reads 4 CONSECUTIVE dram rows (one big
    # descriptor instead of 4): s = q*512 + p*4 + t. The relabeling flows
    # consistently through u/e/softmax/ctx (softmax is order-invariant and
    # every consumer uses the same tiling), so results are unchanged.
    enc_r = enc.rearrange("b (q p t) e -> b p q t e", q=NQ, p=P, t=TQ)
    # U rows paired (consecutive e) to match the fp8-in-u16 transpose:
    # U_sb[p, (kb c), a] = U[kb*256 + 2p + c, a]
    U_r = U_a.rearrange("(kb p c) a -> p kb c a", kb=KB, p=P, c=2)
    # W/ffn contractions relabeled k-major (d = p*8 + k) so each partition
    # reads consecutive dram rows -> 128-descriptor DMAs that don't choke
    # the SWDGE prep ring. The dec/ctx transposes below use matching
    # strided views, so results are unchanged.
    W_r = W_a.rearrange("(p k) a -> p k a", p=P)
    ffn_r = ffn.rearrange("(hf p c) d -> p hf c d", hf=2, p=P)

    with (
        tc.tile_pool(name="weights", bufs=1) as weights,
        tc.tile_pool(name="enc8", bufs=6) as enc8_pool,
        tc.tile_pool(name="encT8", bufs=4) as encT8_pool,
        tc.tile_pool(name="tanhp", bufs=9) as tanh_pool,
        tc.tile_pool(name="rows", bufs=1) as rows,
        tc.tile_pool(name="rows2", bufs=2) as rows2,
        tc.tile_pool(name="rows4", bufs=4) as rows4,
        tc.tile_pool(name="qtiles", bufs=1) as qtiles,
        tc.tile_pool(name="psum_u", bufs=3, space="PSUM") as psum_u,
        tc.tile_pool(name="psum_e", bufs=3, space="PSUM") as psum_e,
        tc.tile_pool(name="psum_c", bufs=1, space="PSUM") as psum_c,
    ):
        # ---------------- Pool-queue loads (device order matters) --------
        # Startup chain to the first tanh: enc(unit0) -> T(unit0) while
        # U8 (direct fp8) and W + wT land -> first tanh ~15us. Units 0/1
        # use separate tiles (dependency tracking is tile-granular).
        # dec rides the SP HWDGE queue as fp32 (no Pool prep, no cast) and
        # is transposed on the idle PE instead of the xbar; one flat [1, B*D]
        # tile (single descriptor, single DMA-window slot) keeps every
        # PE-transpose input at partition 0.
        dec_flat = rows.tile([1, NB * D], F32, tag="dec32")
        nc.sync.dma_start(out=dec_flat, in_=dec[:, :])

        # U loads straight to fp8 (gpsimd DMA cast, no scale): u already
        # tolerates ~3% fp8 noise on enc; the raw-range U quantization adds
        # ~1.2x to that one term and saves the bf16 load + DVE cast from
        # the startup critical path. Per-kb tiles keep deps precise.
        U8k = [weights.tile([P, 2, A], FP8, name=f"U8_{kb}")
               for kb in range(KB)]
        v_sb = weights.tile([P, MA], BF16)

        def load_U(kb):
            # full-a per-kb: partition p reads 2 consecutive dram rows (8KB)
            # per descriptor -> 128 descriptors/DMA (SWDGE-ring friendly)
            nc.gpsimd.dma_start(out=U8k[kb], in_=U_r[:, kb])

        def load_W_half(h):
            W_h = weights.tile([P, KD, N512], BF16, name=f"W_h{h}",
                               tag="Whalf")
            asl = slice(h * N512, (h + 1) * N512)
            for k in (0, 4):
                nc.gpsimd.dma_start(
                    out=W_h[:, k : k + 4, :], in_=W_r[:, k : k + 4, asl]
                )
            return W_h

        # unit_nat[k]: t -> (tile, local_t) natural fp8 enc for unit k
        unit_nat = {}

        def load_enc_pair(pp):
            nat = enc8_pool.tile([P, 2 * TQ, E], FP8, name=f"nat_{pp}",
                                 tag="enc8")
            b, q = divmod(2 * pp, NQ)
            nc.gpsimd.dma_start(out=nat, in_=enc_r[b, :, q : q + 2, :, :])
            unit_nat[2 * pp] = lambda t, nat=nat: (nat, t)
            unit_nat[2 * pp + 1] = lambda t, nat=nat: (nat, TQ + t)

        def load_enc_unit(k):
            nat = qtiles.tile([P, TQ, E], FP8, name=f"natu_{k}",
                              tag=f"enc8u{k}")
            b, q = divmod(k, NQ)
            nc.gpsimd.dma_start(out=nat, in_=enc_r[b, :, q, :, :])
            unit_nat[k] = lambda t, nat=nat: (nat, t)

        # ---------------- small shared tiles ----------------
        # catT[p, c, j] = cat[j, c*128+p] ; c 0..7 dec, 8..15 ctx (bf16,
        # written per-batch from PE transposes of ctx_row).
        catT = weights.tile([P, 2 * KE, NB], BF16)
        ones128 = weights.tile([1, P], BF16)
        nc.vector.memset(ones128, 1.0)
        id1 = weights.tile([1, 1], F32)
        nc.vector.memset(id1, 1.0)
        # dummy activation so the 1.3us LoadActFuncSet runs at t~0 instead
        # of right before the first real tanh
        act_warm = weights.tile([1, 1], F32)
        nc.scalar.activation(act_warm, id1, AF.Tanh)
        # dec transpose on PE, k-major to match W_r's row labeling:
        # catT[p, k, j] = dec[j, p*8+k]; per-row [1,128]->[128,1] transposes
        decT_ps = psum_u.tile([P, KE, NB], F32, tag="u")
        dec_kv = dec_flat.rearrange("o (j p2 k) -> o j k p2", j=NB, k=KD)
        for j in range(NB):
            for k in range(KD):
                nc.tensor.transpose(
                    decT_ps[:, k, j : j + 1], dec_kv[:, j, k, :], id1
                )
        nc.vector.tensor_copy(
            catT[:, 0:KE, :].rearrange("p c j -> p (c j)"),
            decT_ps.rearrange("p c j -> p (c j)"),
        )

        # W streams through ONE half-size tile (a-halves, sequentially):
        # the wT(m0-3) matmuls are emitted BETWEEN the two loads, so the
        # pool's WAR tracking orders the second load after them. Saves
        # 8KB/partition of SBUF (W is dead after startup), which pays for
        # the 4th encT8 transpose buffer.
        wT_ps = psum_c.tile([P, MA, NB], F32, tag="cvec")
        wT = weights.tile([P, MA, NB], F32)

        def emit_w_half(h, W_h):
            for m in range(4 * h, 4 * h + 4):
                for k in range(KD):
                    nc.tensor.matmul(
                        wT_ps[:, m, :],
                        lhsT=W_h[:, k, (m - 4 * h) * P : (m - 4 * h + 1) * P],
                        rhs=catT[:, k, :],
                        start=(k == 0),
                        stop=(k == KD - 1),
                    )
            sl = slice(4 * h, 4 * h + 4)
            nc.vector.tensor_copy(
                wT[:, sl, :].rearrange("p m b -> p (m b)"),
                wT_ps[:, sl, :].rearrange("p m b -> p (m b)"),
            )

        # ---------------- transposes (SP HWDGE queue) ----------------
        # eTp[p, (qq t k), (j c)]: fp8 pairs viewed as u16 through the xbar.
        # rhs_view[k] is a list of (view, t0, tn) segments; view dims are
        # [p, kb, c, t, j] fp8 slices of the transposed result.
        rhs_view = {}

        def emit_T_pair(pp):
            eTp = encT8_pool.tile([P, 8 * TQ, P], U16, name=f"eT_{pp}",
                                  tag="encT8")
            src = unit_nat[2 * pp](0)[0].rearrange(
                "p t e -> p (t e)").bitcast(U16)
            nc.sync.dma_start(out=eTp, in_=src, transpose=True)
            pair_view = eTp[:, :, :].bitcast(FP8).rearrange(
                "p (qq t k) (j c) -> p qq k c t j", qq=2, t=TQ, k=KB, c=2
            )
            rhs_view[2 * pp] = [(pair_view[:, 0], 0, TQ)]
            rhs_view[2 * pp + 1] = [(pair_view[:, 1], 0, TQ)]

        def emit_T_unit(k):
            eTu = qtiles.tile([P, 4 * TQ, P], U16, name=f"eTu_{k}",
                              tag=f"encT8u{k}")
            src = unit_nat[k](0)[0].rearrange("p t e -> p (t e)").bitcast(U16)
            nc.sync.dma_start(out=eTu, in_=src, transpose=True)
            rhs_view[k] = [(eTu[:, :, :].bitcast(FP8).rearrange(
                "p (t k) (j c) -> p k c t j", t=TQ, k=KB, c=2
            ), 0, TQ)]

        load_enc_unit(0)
        emit_T_unit(0)
        for kb in range(KB):
            load_U(kb)
        Wh = load_W_half(0)
        emit_w_half(0, Wh)
        load_enc_unit(1)
        emit_T_unit(1)
        Wh2 = load_W_half(1)
        emit_w_half(1, Wh2)
        nc.gpsimd.dma_start(
            out=v_sb, in_=v_t.rearrange("(m p) one -> p (m one)", p=P)
        )
        load_enc_pair(1)
        # remaining enc pairs + ffn are emitted inside the main loop.
        # NOTE on DMA ordering: the scheduler issues DMAs through a bounded
        # in-flight window in program order, so every DMA/transpose must
        # have its dependencies long-resolved by the time its turn comes;
        # transposes are emitted one pair-slot behind their input load, and
        # all pair tiles are fresh buffers (no WAR waits in the stream).

        # v8[p, mm, j, cc] = v[(2mm+j)*128+p] * 32 fp8, replicated 128 wide
        # (dual-fp8 Ldweights rejects narrow stationaries); the e-matmul
        # output is 128 identical rows of which exp reads row 0.
        v32 = weights.tile([P, MA], F32)
        nc.vector.tensor_scalar_mul(v32, v_sb, V_SCALE)
        zero128 = weights.tile([P, P], F32)
        nc.vector.memset(zero128, 0.0)
        v8 = weights.tile([P, MA // 2, 2, P], FP8)
        for mm in range(MA // 2):
            for j in range(2):
                nc.vector.tensor_scalar_add(
                    v8[:, mm, j, :], zero128, v32[:, 2 * mm + j : 2 * mm + j + 1]
                )

        # ffn_sb[p, hf, c, d] = ffn[hf*1024 + p*8 + c, d] (k-major halves:
        # hf=0 dec rows, hf=1 ctx rows); 4 DMAs of 64 big descriptors
        ffn_sb = weights.tile([P, 2, KD, D], BF16)

        def load_ffn():
            for hf in range(2):
                for c in (0, 4):
                    nc.gpsimd.dma_start(
                        out=ffn_sb[:, hf, c : c + 4, :],
                        in_=ffn_r[:, hf, c : c + 4, :],
                    )


        # ---------------- per-unit / per-batch helpers ----------------
        bst = {}

        def batch_state(b):
            if b not in bst:
                bst[b] = {
                    "expe": rows2.tile([1, S], BF16, name=f"expe_{b}",
                                       tag="expe"),
                    "esum4": rows2.tile([1, NQ], F32, name=f"esum4_{b}",
                                        tag="esum4"),
                    "th2s": {},
                    "e_ps": {},
                }
            return bst[b]

        def build_u_block(k):
            """u matmuls (fp8 DR) + fused tanh (fp8 out) for unit k."""
            b, q = divmod(k, NQ)
            st = batch_state(b)
            th2s = []
            th2 = None
            for m in range(MA):
                u_ps = psum_u.tile([P, SQ], F32, name="u_ps", tag="u")
                for (seg, t0, tn) in rhs_view[k]:
                    for kb in range(KB):
                        nc.tensor.matmul(
                            u_ps[:, t0 * P : (t0 + tn) * P],
                            lhsT=U8k[kb][:, :, m * P : (m + 1) * P],
                            rhs=seg[:, kb],
                            start=(kb == 0),
                            stop=(kb == KB - 1),
                            perf_mode=DR,
                        )
                if m % 2 == 0:
                    th2 = tanh_pool.tile([P, 2, SQ], FP8, name="th2", tag="th")
                nc.scalar.activation(
                    th2[:, m % 2, :], u_ps, AF.Tanh,
                    bias=wT[:, m, b : b + 1],
                    scale=1.0 / U_SCALE,
                )
                if m % 2 == 1:
                    th2s.append(th2)
            st["th2s"][q] = th2s

        def emit_e_exp(k):
            """e-matmuls (fp8 DR over tanh pairs) + exp for unit k; emitted
            one unit late so PE/ACT never wait on the tanh lag."""
            b, q = divmod(k, NQ)
            st = batch_state(b)
            e_ps = psum_e.tile([P, SQ], F32, name=f"e_ps_{k}", tag="eps")
            for mm, t2 in enumerate(st["th2s"].pop(q)):
                nc.tensor.matmul(
                    e_ps,
                    lhsT=v8[:, mm, :, :],
                    rhs=t2,
                    start=(mm == 0),
                    stop=(mm == MA // 2 - 1),
                    perf_mode=DR,
                )
            # e_ps holds 32*e (v8 scaling), undone by the exp scale
            nc.scalar.activation(
                st["expe"][:, q * SQ : (q + 1) * SQ],
                e_ps[0:1, :],
                AF.Exp,
                scale=1.0 / V_SCALE,
                accum_out=st["esum4"][:, q : q + 1],
            )

        def emit_rep(b):
            """PE replicate-matmuls: expe8_rep[p, tg, :] = expe[tg*128+p]
            (fp8, 128-wide) in 4 single-bank chunks, DVE-copied to sbuf.
            rep8b holds the fp8 cast RESIDUAL (expe - fp8(expe)): the ctx
            matmul accumulates both, wiping the softmax-weight quantization
            error (~30% of ctx's error budget) for one extra DVE op and 16
            extra 107ns DR matmuls per batch."""
            alloc_rep(b)
            emit_rep_chunks(b, range(4))
            emit_sums(b)

        def alloc_rep(b):
            st = batch_state(b)
            st["rep8"] = rows2.tile([P, ST, P], FP8, name=f"rep8_{b}",
                                    tag="rep8")
            st["rep8b"] = rows2.tile([P, ST, P], FP8, name=f"rep8b_{b}",
                                     tag="rep8b")

        def emit_rep_chunks(b, chunks):
            st = batch_state(b)
            rep8, rep8b = st["rep8"], st["rep8b"]
            for c in chunks:
                rep_ps = psum_e.tile([P, 4 * P], F32, name=f"rep_ps_{b}_{c}",
                                     tag="eps")
                for t in range(4):
                    tg = c * 4 + t
                    nc.tensor.matmul(
                        rep_ps[:, t * P : (t + 1) * P],
                        lhsT=st["expe"][:, tg * P : (tg + 1) * P],
                        rhs=ones128,
                        start=True,
                        stop=True,
                    )
                sl = rep8[:, 4 * c : 4 * (c + 1), :].rearrange(
                    "p t j -> p (t j)")
                nc.vector.tensor_copy(sl, rep_ps)
                nc.vector.tensor_tensor(
                    rep8b[:, 4 * c : 4 * (c + 1), :].rearrange(
                        "p t j -> p (t j)"),
                    rep_ps, sl, mybir.AluOpType.subtract,
                )

        def emit_sums(b):
            st = batch_state(b)
            esum = rows2.tile([1, 1], F32, name=f"esumt_{b}", tag="esumt")
            nc.vector.tensor_reduce(esum, st["esum4"], mybir.AxisListType.X,
                                    mybir.AluOpType.add)
            rsum = rows2.tile([1, 1], F32, name=f"rsum_{b}", tag="rsum")
            nc.vector.reciprocal(rsum, esum)
            st["rsum"] = rsum

        def emit_ctx(b):
            """fp8 DoubleRow ctx matmuls + scale/copy-out (DVE)."""
            emit_ctx_mms(b, range(ST // 2))
            emit_ctx_fin(b)

        def emit_ctx_mms(b, urange):
            st = batch_state(b)
            if "ctx_ps" not in st:
                st["ctx_ps"] = psum_c.tile([P, 2, N512], F32,
                                           name=f"ctx_ps_{b}", tag="cvec")
            ctx_ps = st["ctx_ps"]
            for n in range(2):
                for h, rep in enumerate((st["rep8"], st["rep8b"])):
                    for u in urange:
                        q, t = divmod(2 * u, NQ)
                        nat, lt = unit_nat[b * NQ + q](t)
                        nc.tensor.matmul(
                            ctx_ps[:, n, :],
                            lhsT=rep[:, 2 * u : 2 * u + 2, :],
                            rhs=nat[:, lt : lt + 2,
                                    n * N512 : (n + 1) * N512],
                            start=(h == 0 and u == 0),
                            stop=(h == 1 and u == ST // 2 - 1),
                            perf_mode=DR,
                            skip_group_check=True,
                        )

        def emit_ctx_fin(b):
            st = batch_state(b)
            ctx_ps = st["ctx_ps"]
            ctx_row = rows4.tile([1, E], F32, name=f"ctx_row_{b}",
                                 tag="ctx_row")
            st["ctx_row"] = ctx_row
            nc.vector.tensor_scalar_mul(
                ctx_row, ctx_ps[0:1, :, :].rearrange("o n f -> o (n f)"),
                st["rsum"],
            )
            # ctx column of catT via PE transposes (keeps the tail off the
            # xbar and the SP queue)
            # k-major strided views to match ffn_r's ctx-half row labeling
            ctx_kv = ctx_row.rearrange("o (p2 c) -> o c p2", c=KE)
            ctxT_ps = psum_e.tile([P, KE], F32, name=f"ctxT_{b}", tag="eps")
            for c in range(KE):
                nc.tensor.transpose(
                    ctxT_ps[:, c : c + 1], ctx_kv[:, c, :], id1
                )
            nc.vector.tensor_copy(
                catT[:, KE : 2 * KE, b : b + 1].rearrange("p c j -> p (c j)"),
                ctxT_ps,
            )

        # ---------------- main pipeline ----------------
        # Unit k's u-block is emitted at slot k; e/exp of k-1 after it;
        # rep(b-1) after slot 4b+1's u-block; ctx(b-1) after slot 4b+2's
        # (so the DVE rep8 copies overlap u(4b+2)'s execution).
        for k in range(NU):
            if k % 2 == 0 and k // 2 + 2 < NP:
                load_enc_pair(k // 2 + 2)
            if k % 2 == 0 and 1 <= k // 2 + 1 < NP:
                emit_T_pair(k // 2 + 1)
            if k == 12:
                load_ffn()
            build_u_block(k)
            if k >= 1:
                emit_e_exp(k - 1)
            if k >= 5 and k % NQ == 1:
                emit_rep(k // NQ - 1)
            if k >= 6 and k % NQ == 2:
                emit_ctx(k // NQ - 1)
            # last batch: front-run softmax replication + the q0/q1 half of
            # the ctx matmuls so only ~half the chain trails the last tanh
            if k == NU - 2:
                alloc_rep(NB - 1)
                emit_rep_chunks(NB - 1, (0, 1))
            if k == NU - 1:
                emit_rep_chunks(NB - 1, (2,))
                emit_ctx_mms(NB - 1, (0, 1, 2, 3))
        emit_e_exp(NU - 1)
        emit_rep_chunks(NB - 1, (3,))
        emit_sums(NB - 1)
        emit_ctx_mms(NB - 1, (4, 5, 6, 7))
        emit_ctx_fin(NB - 1)

        # ---------------- final ffn (all batches at once) ----------------
        out_ps = psum_c.tile([NB, D], F32, tag="cvec")
        for hf in range(2):
            for c in range(KD):
                for n in range(2):
                    nc.tensor.matmul(
                        out_ps[:, n * N512 : (n + 1) * N512],
                        lhsT=catT[:, hf * KE + c, :],
                        rhs=ffn_sb[:, hf, c, n * N512 : (n + 1) * N512],
                        start=(hf == 0 and c == 0),
                        stop=(hf == 1 and c == KD - 1),
                    )
        out_sb = weights.tile([NB, D], F32)
        nc.scalar.activation(out_sb, out_ps, AF.Tanh)
        nc.scalar.dma_start(out=out[:, :], in_=out_sb)
        # ctx stores last: tiny DMAs whose deps resolved long ago, so they
        # never head-block the issue window mid-stream
        for b in range(NB):
            nc.scalar.dma_start(out=ctx_out[b : b + 1, :],
                                in_=bst[b]["ctx_row"])


_NC_CACHE = None


def _get_nc(repeat=1):
    global _NC_CACHE
    if repeat != 1:
        nc = bacc.Bacc(None, target_bir_lowering=False)
        with tile.TileContext(nc) as tc:
            _build_kernel_body(tc, repeat=repeat)
        nc.compile()
        return nc
    if _NC_CACHE is None:
        nc = bacc.Bacc(None, target_bir_lowering=False)
        with tile.TileContext(nc) as tc:
            _build_kernel_body(tc)
        nc.compile()
        _NC_CACHE = nc
    return _NC_CACHE


def kernel(encoder_hidden_states, decoder_hidden_state, U_a, W_a, v_t, ffn,
           _trace=False):
    enc = np.ascontiguousarray(np.asarray(encoder_hidden_states, dtype=np.float32))
    dec = np.ascontiguousarray(
        np.asarray(decoder_hidden_state, dtype=np.float32).reshape(B, D)
    )
    U = np.ascontiguousarray(np.asarray(U_a, dtype=np.float32))
    W = np.ascontiguousarray(np.asarray(W_a, dtype=np.float32))
    v = np.ascontiguousarray(np.asarray(v_t, dtype=np.float32))
    F = np.ascontiguousarray(np.asarray(ffn, dtype=np.float32))

    nc = _get_nc()
    in_maps = []
    for c in range(NCORES):
        sl = slice(c * NB, (c + 1) * NB)
        in_maps.append(
            {
                "enc": enc[sl],
                "dec": dec[sl],
                "U_a": U,
                "W_a": W,
                "v_t": v,
                "ffn": F,
            }
        )
    res = run_bass_kernel_spmd(nc, in_maps, core_ids=list(range(NCORES)),
                               trace=_trace)

    output = np.empty((B, 1, D), dtype=np.float32)
    context = np.empty((B, 1, E), dtype=np.float32)
    for c in range(NCORES):
        sl = slice(c * NB, (c + 1) * NB)
        output[sl, 0, :] = res.results[c]["out"]
        context[sl, 0, :] = res.results[c]["ctx_out"]
    if _trace:
        return (output, context), res
    return (output, context)


if __name__ == "__main__":
    import reference

    inputs = {k: np.asarray(v) for k, v in reference.setup_inputs().items()}
    (o, c) = kernel(**inputs)
    print("output", o.shape, o.dtype, "context", c.shape, c.dtype)


# revision 122
# speedup vs baseline: 1.0475x; 1.0340x over previous
"""Bahdanau additive attention kernel for Trainium2 (8 NeuronCores, SPMD).

Problem: B=32, S=2048, ENC=DEC=ATT=1024 (fp32 inputs)
  u = enc @ U_a                [B,S,A]
  w = dec @ W_a                [B,1,A]
  e = tanh(w + u) @ v_t        [B,S,1]
  align = softmax(e, axis=1)
  context = align^T @ enc      [B,1,E]
  output = tanh([dec, context] @ ffn)   [B,1,D]
  returns (output, context)

Sharding: data-parallel over batch, 4 batches per core, weights replicated.

v2 design (vs v1): enc and U are DMA-cast fp32->fp8 directly (no bf16
copies, no DVE casts); the ctx matmul runs fp8 DoubleRow against the
same fp8 enc tiles, with softmax weights replicated 128-wide (PE
replicate-matmuls + DVE fp8 copies) and a second fp8 RESIDUAL pass
(rep8b) that cancels the weight-quantization error; dec/ctx transposes
for the ffn cat run on the PE (k-major relabeled to match big-descriptor
W/ffn loads) so the ACT engine streams pure tanh+exp; e-matmuls/exp of
unit k are emitted after unit k+1's u-block so PE never waits on the
tanh lag; the last batch's softmax/ctx work is front-run so only half
the chain trails the final tanh.

Per-core engine budget (TimelineSim, 145.3us total): ACT ~93us (128
tanh + 16 exp), PE ~95us (512 u-MM fp8 DR + e/rep/ctx x2/ffn/w), DMA
device ~73us (enc fp8 loads 23 + xbar transposes 29 + U/W/ffn 20).
Scheduler constraint that shaped the layout: DMAs issue through a
small in-flight window in the scheduler's global order with ~2us of
completion latency per hop, so every DMA's deps must be long-resolved
at its turn (transposes trail their input load by one pair slot, all
pair tiles are fresh buffers, stores go last).
"""

import numpy as np
import ml_dtypes

import concourse.bass as bass
import concourse.mybir as mybir
import concourse.tile as tile
from concourse import bacc
from concourse.bass_utils import run_bass_kernel_spmd

F32 = mybir.dt.float32
BF16 = mybir.dt.bfloat16
FP8 = mybir.dt.float8e4
U16 = mybir.dt.uint16
AF = mybir.ActivationFunctionType
DR = mybir.MatmulPerfMode.DoubleRow

U_SCALE = 1.0   # U loaded as raw fp8 (no scale)
V_SCALE = 32.0

B, S, E, A, D = 32, 2048, 1024, 1024, 1024
NCORES = 8
NB = B // NCORES          # 4 batches per core
P = 128
KE = E // P               # 8 e-chunks (128 each)
KB = 4                    # e-pair blocks (256 e-values each) for DoubleRow
MA = A // P               # 8 output chunks over att dim
KD = D // P               # 8 contraction chunks over dec dim
ST = S // P               # 16 s-tiles per batch
NQ = 4                    # units per batch
TQ = ST // NQ             # 4 s-tiles per unit (512 seq)
SQ = TQ * P               # 512 seq per unit
N512 = 512
NU = NB * NQ              # 16 units per core
NP = NU // 2              # 8 pair (2-unit) load/transpose groups


def _build_kernel_body(tc, repeat=1):
    nc = tc.nc
    enc = nc.dram_tensor("enc", [NB, S, E], F32, kind="ExternalInput")
    dec = nc.dram_tensor("dec", [NB, D], F32, kind="ExternalInput")
    U_a = nc.dram_tensor("U_a", [E, A], F32, kind="ExternalInput")
    W_a = nc.dram_tensor("W_a", [D, A], F32, kind="ExternalInput")
    v_t = nc.dram_tensor("v_t", [A, 1], F32, kind="ExternalInput")
    ffn = nc.dram_tensor("ffn", [D + E, D], F32, kind="ExternalInput")
    out = nc.dram_tensor("out", [NB, D], F32, kind="ExternalOutput")
    ctx_out = nc.dram_tensor("ctx_out", [NB, E], F32, kind="ExternalOutput")
    for _ in range(repeat):
        _build_once(tc, enc, dec, U_a, W_a, v_t, ffn, out, ctx_out)


def _build_once(tc, enc, dec, U_a, W_a, v_t, ffn, out, ctx_out):
    nc = tc.nc
    # s relabeled so each partition reads 4 CONSECUTIVE dram rows (one big
    # descriptor instead of 4): s = q*512 + p*4 + t. The relabeling flows
    # consistently through u/e/softmax/ctx (softmax is order-invariant and
    # every consumer uses the same tiling), so results are unchanged.
    enc_r = enc.rearrange("b (q p t) e -> b p q t e", q=NQ, p=P, t=TQ)
    # U rows paired (consecutive e) to match the fp8-in-u16 transpose:
    # U_sb[p, (kb c), a] = U[kb*256 + 2p + c, a]
    U_r = U_a.rearrange("(kb p c) a -> p kb c a", kb=KB, p=P, c=2)
    # W/ffn contractions relabeled k-major (d = p*8 + k) so each partition
    # reads consecutive dram rows -> 128-descriptor DMAs that don't choke
    # the SWDGE prep ring. The dec/ctx transposes below use matching
    # strided views, so results are unchanged.
    W_r = W_a.rearrange("(p k) a -> p k a", p=P)
    ffn_r = ffn.rearrange("(hf p c) d -> p hf c d", hf=2, p=P)

    with (
        tc.tile_pool(name="weights", bufs=1) as weights,
        tc.tile_pool(name="enc8", bufs=6) as enc8_pool,
        tc.tile_pool(name="encT8", bufs=4) as encT8_pool,
        tc.tile_pool(name="tanhp", bufs=9) as tanh_pool,
        tc.tile_pool(name="rows", bufs=1) as rows,
        tc.tile_pool(name="rows2", bufs=2) as rows2,
        tc.tile_pool(name="rows4", bufs=4) as rows4,
        tc.tile_pool(name="qtiles", bufs=1) as qtiles,
        tc.tile_pool(name="psum_u", bufs=3, space="PSUM") as psum_u,
        tc.tile_pool(name="psum_e", bufs=3, space="PSUM") as psum_e,
        tc.tile_pool(name="psum_c", bufs=1, space="PSUM") as psum_c,
    ):
        # ---------------- Pool-queue loads (device order matters) --------
        # Startup chain to the first tanh: enc(pair0) -> T(pair0) while
        # U8 (direct fp8) and W halves + wT land -> first tanh ~21us.
        # dec rides the SP HWDGE queue as fp32 (no Pool prep, no cast) and
        # is transposed on the idle PE instead of the xbar; one flat [1, B*D]
        # tile (single descriptor, single DMA-window slot) keeps every
        # PE-transpose input at partition 0.
        dec_flat = rows.tile([1, NB * D], F32, tag="dec32")
        nc.sync.dma_start(out=dec_flat, in_=dec[:, :])

        # U loads straight to fp8 (gpsimd DMA cast, no scale): u already
        # tolerates ~3% fp8 noise on enc; the raw-range U quantization adds
        # ~1.2x to that one term and saves the bf16 load + DVE cast from
        # the startup critical path. Per-kb tiles keep deps precise.
        U8k = [weights.tile([P, 2, A], FP8, name=f"U8_{kb}")
               for kb in range(KB)]
        v_sb = weights.tile([P, MA], BF16)

        def load_U(kb):
            # full-a per-kb: partition p reads 2 consecutive dram rows (8KB)
            # per descriptor -> 128 descriptors/DMA (SWDGE-ring friendly)
            nc.gpsimd.dma_start(out=U8k[kb], in_=U_r[:, kb])

        def load_W_half(h):
            W_h = weights.tile([P, KD, N512], BF16, name=f"W_h{h}",
                               tag="Whalf")
            asl = slice(h * N512, (h + 1) * N512)
            for k in (0, 4):
                nc.gpsimd.dma_start(
                    out=W_h[:, k : k + 4, :], in_=W_r[:, k : k + 4, asl]
                )
            return W_h

        # unit_nat[k]: t -> (tile, local_t) natural fp8 enc for unit k
        unit_nat = {}

        def load_enc_pair(pp):
            nat = enc8_pool.tile([P, 2 * TQ, E], FP8, name=f"nat_{pp}",
                                 tag="enc8")
            b, q = divmod(2 * pp, NQ)
            nc.gpsimd.dma_start(out=nat, in_=enc_r[b, :, q : q + 2, :, :])
            unit_nat[2 * pp] = lambda t, nat=nat: (nat, t)
            unit_nat[2 * pp + 1] = lambda t, nat=nat: (nat, TQ + t)

        def load_enc_unit(k):
            nat = qtiles.tile([P, TQ, E], FP8, name=f"natu_{k}",
                              tag=f"enc8u{k}")
            b, q = divmod(k, NQ)
            nc.gpsimd.dma_start(out=nat, in_=enc_r[b, :, q, :, :])
            unit_nat[k] = lambda t, nat=nat: (nat, t)

        # ---------------- small shared tiles ----------------
        # catT[p, c, j] = cat[j, c*128+p] ; c 0..7 dec, 8..15 ctx (bf16,
        # written per-batch from PE transposes of ctx_row).
        catT = weights.tile([P, 2 * KE, NB], BF16)
        ones128 = weights.tile([1, P], BF16)
        nc.vector.memset(ones128, 1.0)
        id1 = weights.tile([1, 1], F32)
        nc.vector.memset(id1, 1.0)
        # dummy activation so the 1.3us LoadActFuncSet runs at t~0 instead
        # of right before the first real tanh
        act_warm = weights.tile([1, 1], F32)
        nc.scalar.activation(act_warm, id1, AF.Tanh)
        # dec transpose on PE, k-major to match W_r's row labeling:
        # catT[p, k, j] = dec[j, p*8+k]; per-row [1,128]->[128,1] transposes
        decT_ps = psum_u.tile([P, KE, NB], F32, tag="u")
        dec_kv = dec_flat.rearrange("o (j p2 k) -> o j k p2", j=NB, k=KD)
        for j in range(NB):
            for k in range(KD):
                nc.tensor.transpose(
                    decT_ps[:, k, j : j + 1], dec_kv[:, j, k, :], id1
                )
        nc.vector.tensor_copy(
            catT[:, 0:KE, :].rearrange("p c j -> p (c j)"),
            decT_ps.rearrange("p c j -> p (c j)"),
        )

        # W streams through ONE half-size tile (a-halves, sequentially):
        # the wT(m0-3) matmuls are emitted BETWEEN the two loads, so the
        # pool's WAR tracking orders the second load after them. Saves
        # 8KB/partition of SBUF (W is dead after startup), which pays for
        # the 4th encT8 transpose buffer.
        wT_ps = psum_c.tile([P, MA, NB], F32, tag="cvec")
        wT = weights.tile([P, MA, NB], F32)

        def emit_w_half(h, W_h):
            for m in range(4 * h, 4 * h + 4):
                for k in range(KD):
                    nc.tensor.matmul(
                        wT_ps[:, m, :],
                        lhsT=W_h[:, k, (m - 4 * h) * P : (m - 4 * h + 1) * P],
                        rhs=catT[:, k, :],
                        start=(k == 0),
                        stop=(k == KD - 1),
                    )
            sl = slice(4 * h, 4 * h + 4)
            nc.vector.tensor_copy(
                wT[:, sl, :].rearrange("p m b -> p (m b)"),
                wT_ps[:, sl, :].rearrange("p m b -> p (m b)"),
            )

        # ---------------- transposes (SP HWDGE queue) ----------------
        # eTp[p, (qq t k), (j c)]: fp8 pairs viewed as u16 through the xbar.
        # rhs_view[k] is a list of (view, t0, tn) segments; view dims are
        # [p, kb, c, t, j] fp8 slices of the transposed result.
        rhs_view = {}

        def emit_T_pair(pp):
            eTp = encT8_pool.tile([P, 8 * TQ, P], U16, name=f"eT_{pp}",
                                  tag="encT8")
            src = unit_nat[2 * pp](0)[0].rearrange(
                "p t e -> p (t e)").bitcast(U16)
            nc.sync.dma_start(out=eTp, in_=src, transpose=True)
            pair_view = eTp[:, :, :].bitcast(FP8).rearrange(
                "p (qq t k) (j c) -> p qq k c t j", qq=2, t=TQ, k=KB, c=2
            )
            rhs_view[2 * pp] = [(pair_view[:, 0], 0, TQ)]
            rhs_view[2 * pp + 1] = [(pair_view[:, 1], 0, TQ)]

        def emit_T_unit(k):
            eTu = qtiles.tile([P, 4 * TQ, P], U16, name=f"eTu_{k}",
                              tag=f"encT8u{k}")
            src = unit_nat[k](0)[0].rearrange("p t e -> p (t e)").bitcast(U16)
            nc.sync.dma_start(out=eTu, in_=src, transpose=True)
            rhs_view[k] = [(eTu[:, :, :].bitcast(FP8).rearrange(
                "p (t k) (j c) -> p k c t j", t=TQ, k=KB, c=2
            ), 0, TQ)]

        # pair 0 uses the same pair-granular load/transpose as pairs 1-7:
        # per-unit splitting looked better for dependency precision, but two
        # extra items in the scheduler's bounded DMA window cost far more
        # (the late unit-1 transpose stalled the second W half by ~6us)
        load_enc_pair(0)
        emit_T_pair(0)
        for kb in range(KB):
            load_U(kb)
        Wh = load_W_half(0)
        emit_w_half(0, Wh)
        Wh2 = load_W_half(1)
        emit_w_half(1, Wh2)
        nc.gpsimd.dma_start(
            out=v_sb, in_=v_t.rearrange("(m p) one -> p (m one)", p=P)
        )
        load_enc_pair(1)
        # remaining enc pairs + ffn are emitted inside the main loop.
        # NOTE on DMA ordering: the scheduler issues DMAs through a bounded
        # in-flight window in program order, so every DMA/transpose must
        # have its dependencies long-resolved by the time its turn comes;
        # transposes are emitted one pair-slot behind their input load, and
        # all pair tiles are fresh buffers (no WAR waits in the stream).

        # v8[p, mm, j, cc] = v[(2mm+j)*128+p] * 32 fp8, replicated 128 wide
        # (dual-fp8 Ldweights rejects narrow stationaries); the e-matmul
        # output is 128 identical rows of which exp reads row 0.
        v32 = weights.tile([P, MA], F32)
        nc.vector.tensor_scalar_mul(v32, v_sb, V_SCALE)
        zero128 = weights.tile([P, P], F32)
        nc.vector.memset(zero128, 0.0)
        v8 = weights.tile([P, MA // 2, 2, P], FP8)
        for mm in range(MA // 2):
            for j in range(2):
                nc.vector.tensor_scalar_add(
                    v8[:, mm, j, :], zero128, v32[:, 2 * mm + j : 2 * mm + j + 1]
                )

        # ffn_sb[p, hf, c, d] = ffn[hf*1024 + p*8 + c, d] (k-major halves:
        # hf=0 dec rows, hf=1 ctx rows); 4 DMAs of 64 big descriptors
        ffn_sb = weights.tile([P, 2, KD, D], BF16)

        def load_ffn():
            # 1.46us chunks: late transposes contend with ffn on the DMA
            # engines, and a blocked transpose waits at most one chunk
            for hf in range(2):
                for c in (0, 2, 4, 6):
                    nc.gpsimd.dma_start(
                        out=ffn_sb[:, hf, c : c + 2, :],
                        in_=ffn_r[:, hf, c : c + 2, :],
                    )


        # ---------------- per-unit / per-batch helpers ----------------
        bst = {}

        def batch_state(b):
            if b not in bst:
                bst[b] = {
                    "expe": rows2.tile([1, S], BF16, name=f"expe_{b}",
                                       tag="expe"),
                    "esum4": rows2.tile([1, NQ], F32, name=f"esum4_{b}",
                                        tag="esum4"),
                    "th2s": {},
                    "e_ps": {},
                }
            return bst[b]

        def build_u_block(k):
            """u matmuls (fp8 DR) + fused tanh (fp8 out) for unit k."""
            b, q = divmod(k, NQ)
            st = batch_state(b)
            th2s = []
            th2 = None
            for m in range(MA):
                u_ps = psum_u.tile([P, SQ], F32, name="u_ps", tag="u")
                for (seg, t0, tn) in rhs_view[k]:
                    for kb in range(KB):
                        nc.tensor.matmul(
                            u_ps[:, t0 * P : (t0 + tn) * P],
                            lhsT=U8k[kb][:, :, m * P : (m + 1) * P],
                            rhs=seg[:, kb],
                            start=(kb == 0),
                            stop=(kb == KB - 1),
                            perf_mode=DR,
                        )
                if m % 2 == 0:
                    th2 = tanh_pool.tile([P, 2, SQ], FP8, name="th2", tag="th")
                nc.scalar.activation(
                    th2[:, m % 2, :], u_ps, AF.Tanh,
                    bias=wT[:, m, b : b + 1],
                    scale=1.0 / U_SCALE,
                )
                if m % 2 == 1:
                    th2s.append(th2)
            st["th2s"][q] = th2s

        def emit_e_exp(k):
            """e-matmuls (fp8 DR over tanh pairs) + exp for unit k; emitted
            one unit late so PE/ACT never wait on the tanh lag."""
            b, q = divmod(k, NQ)
            st = batch_state(b)
            e_ps = psum_e.tile([P, SQ], F32, name=f"e_ps_{k}", tag="eps")
            for mm, t2 in enumerate(st["th2s"].pop(q)):
                nc.tensor.matmul(
                    e_ps,
                    lhsT=v8[:, mm, :, :],
                    rhs=t2,
                    start=(mm == 0),
                    stop=(mm == MA // 2 - 1),
                    perf_mode=DR,
                )
            # e_ps holds 32*e (v8 scaling), undone by the exp scale
            nc.scalar.activation(
                st["expe"][:, q * SQ : (q + 1) * SQ],
                e_ps[0:1, :],
                AF.Exp,
                scale=1.0 / V_SCALE,
                accum_out=st["esum4"][:, q : q + 1],
            )

        def emit_rep(b):
            """PE replicate-matmuls: expe8_rep[p, tg, :] = expe[tg*128+p]
            (fp8, 128-wide) in 4 single-bank chunks, DVE-copied to sbuf.
            rep8b holds the fp8 cast RESIDUAL (expe - fp8(expe)): the ctx
            matmul accumulates both, wiping the softmax-weight quantization
            error (~30% of ctx's error budget) for one extra DVE op and 16
            extra 107ns DR matmuls per batch."""
            alloc_rep(b)
            emit_rep_chunks(b, range(4))
            emit_sums(b)

        def alloc_rep(b):
            st = batch_state(b)
            st["rep8"] = rows2.tile([P, ST, P], FP8, name=f"rep8_{b}",
                                    tag="rep8")
            st["rep8b"] = rows2.tile([P, ST, P], FP8, name=f"rep8b_{b}",
                                     tag="rep8b")

        def emit_rep_chunks(b, chunks):
            st = batch_state(b)
            rep8, rep8b = st["rep8"], st["rep8b"]
            for c in chunks:
                rep_ps = psum_e.tile([P, 4 * P], F32, name=f"rep_ps_{b}_{c}",
                                     tag="eps")
                for t in range(4):
                    tg = c * 4 + t
                    nc.tensor.matmul(
                        rep_ps[:, t * P : (t + 1) * P],
                        lhsT=st["expe"][:, tg * P : (tg + 1) * P],
                        rhs=ones128,
                        start=True,
                        stop=True,
                    )
                sl = rep8[:, 4 * c : 4 * (c + 1), :].rearrange(
                    "p t j -> p (t j)")
                nc.vector.tensor_copy(sl, rep_ps)
                nc.vector.tensor_tensor(
                    rep8b[:, 4 * c : 4 * (c + 1), :].rearrange(
                        "p t j -> p (t j)"),
                    rep_ps, sl, mybir.AluOpType.subtract,
                )

        def emit_sums(b):
            st = batch_state(b)
            esum = rows2.tile([1, 1], F32, name=f"esumt_{b}", tag="esumt")
            nc.vector.tensor_reduce(esum, st["esum4"], mybir.AxisListType.X,
                                    mybir.AluOpType.add)
            rsum = rows2.tile([1, 1], F32, name=f"rsum_{b}", tag="rsum")
            nc.vector.reciprocal(rsum, esum)
            st["rsum"] = rsum

        def emit_ctx(b):
            """fp8 DoubleRow ctx matmuls + scale/copy-out (DVE)."""
            emit_ctx_mms(b, range(ST // 2))
            emit_ctx_fin(b)

        def emit_ctx_mms(b, urange):
            st = batch_state(b)
            if "ctx_ps" not in st:
                st["ctx_ps"] = psum_c.tile([P, 2, N512], F32,
                                           name=f"ctx_ps_{b}", tag="cvec")
            ctx_ps = st["ctx_ps"]
            for n in range(2):
                for h, rep in enumerate((st["rep8"], st["rep8b"])):
                    for u in urange:
                        q, t = divmod(2 * u, NQ)
                        nat, lt = unit_nat[b * NQ + q](t)
                        nc.tensor.matmul(
                            ctx_ps[:, n, :],
                            lhsT=rep[:, 2 * u : 2 * u + 2, :],
                            rhs=nat[:, lt : lt + 2,
                                    n * N512 : (n + 1) * N512],
                            start=(h == 0 and u == 0),
                            stop=(h == 1 and u == ST // 2 - 1),
                            perf_mode=DR,
                            skip_group_check=True,
                        )

        def emit_ctx_fin(b):
            st = batch_state(b)
            ctx_ps = st["ctx_ps"]
            ctx_row = rows4.tile([1, E], F32, name=f"ctx_row_{b}",
                                 tag="ctx_row")
            st["ctx_row"] = ctx_row
            nc.vector.tensor_scalar_mul(
                ctx_row, ctx_ps[0:1, :, :].rearrange("o n f -> o (n f)"),
                st["rsum"],
            )
            # ctx column of catT via PE transposes (keeps the tail off the
            # xbar and the SP queue)
            # k-major strided views to match ffn_r's ctx-half row labeling
            ctx_kv = ctx_row.rearrange("o (p2 c) -> o c p2", c=KE)
            ctxT_ps = psum_e.tile([P, KE], F32, name=f"ctxT_{b}", tag="eps")
            for c in range(KE):
                nc.tensor.transpose(
                    ctxT_ps[:, c : c + 1], ctx_kv[:, c, :], id1
                )
            nc.vector.tensor_copy(
                catT[:, KE : 2 * KE, b : b + 1].rearrange("p c j -> p (c j)"),
                ctxT_ps,
            )

        # ---------------- main pipeline ----------------
        # Unit k's u-block is emitted at slot k; e/exp of k-1 after it;
        # rep(b-1) after slot 4b+1's u-block; ctx(b-1) after slot 4b+2's
        # (so the DVE rep8 copies overlap u(4b+2)'s execution).
        for k in range(NU):
            if k % 2 == 0 and k // 2 + 2 < NP:
                load_enc_pair(k // 2 + 2)
            if k % 2 == 0 and 1 <= k // 2 + 1 < NP:
                emit_T_pair(k // 2 + 1)
            if k == 12:
                load_ffn()
            build_u_block(k)
            if k >= 1:
                emit_e_exp(k - 1)
            if k >= 5 and k % NQ == 1:
                emit_rep(k // NQ - 1)
            if k >= 6 and k % NQ == 2:
                emit_ctx(k // NQ - 1)
            # last batch: front-run softmax replication + the q0/q1 half of
            # the ctx matmuls so only ~half the chain trails the last tanh
            if k == NU - 2:
                alloc_rep(NB - 1)
                emit_rep_chunks(NB - 1, (0, 1))
            if k == NU - 1:
                emit_rep_chunks(NB - 1, (2,))
                emit_ctx_mms(NB - 1, (0, 1, 2, 3))
        emit_e_exp(NU - 1)
        emit_rep_chunks(NB - 1, (3,))
        emit_sums(NB - 1)
        emit_ctx_mms(NB - 1, (4, 5, 6, 7))
        emit_ctx_fin(NB - 1)

        # ---------------- final ffn (all batches at once) ----------------
        out_ps = psum_c.tile([NB, D], F32, tag="cvec")
        for hf in range(2):
            for c in range(KD):
                for n in range(2):
                    nc.tensor.matmul(
                        out_ps[:, n * N512 : (n + 1) * N512],
                        lhsT=catT[:, hf * KE + c, :],
                        rhs=ffn_sb[:, hf, c, n * N512 : (n + 1) * N512],
                        start=(hf == 0 and c == 0),
                        stop=(hf == 1 and c == KD - 1),
                    )
        out_sb = weights.tile([NB, D], F32)
        nc.scalar.activation(out_sb, out_ps, AF.Tanh)
        nc.scalar.dma_start(out=out[:, :], in_=out_sb)
        # ctx stores last: tiny DMAs whose deps resolved long ago, so they
        # never head-block the issue window mid-stream
        for b in range(NB):
            nc.scalar.dma_start(out=ctx_out[b : b + 1, :],
                                in_=bst[b]["ctx_row"])


_NC_CACHE = None


def _get_nc(repeat=1):
    global _NC_CACHE
    if repeat != 1:
        nc = bacc.Bacc(None, target_bir_lowering=False)
        with tile.TileContext(nc) as tc:
            _build_kernel_body(tc, repeat=repeat)
        nc.compile()
        return nc
    if _NC_CACHE is None:
        nc = bacc.Bacc(None, target_bir_lowering=False)
        with tile.TileContext(nc) as tc:
            _build_kernel_body(tc)
        nc.compile()
        _NC_CACHE = nc
    return _NC_CACHE


def kernel(encoder_hidden_states, decoder_hidden_state, U_a, W_a, v_t, ffn,
           _trace=False):
    enc = np.ascontiguousarray(np.asarray(encoder_hidden_states, dtype=np.float32))
    dec = np.ascontiguousarray(
        np.asarray(decoder_hidden_state, dtype=np.float32).reshape(B, D)
    )
    U = np.ascontiguousarray(np.asarray(U_a, dtype=np.float32))
    W = np.ascontiguousarray(np.asarray(W_a, dtype=np.float32))
    v = np.ascontiguousarray(np.asarray(v_t, dtype=np.float32))
    F = np.ascontiguousarray(np.asarray(ffn, dtype=np.float32))

    nc = _get_nc()
    in_maps = []
    for c in range(NCORES):
        sl = slice(c * NB, (c + 1) * NB)
        in_maps.append(
            {
                "enc": enc[sl],
                "dec": dec[sl],
                "U_a": U,
                "W_a": W,
                "v_t": v,
                "ffn": F,
            }
        )
    res = run_bass_kernel_spmd(nc, in_maps, core_ids=list(range(NCORES)),
                               trace=_trace)

    output = np.empty((B, 1, D), dtype=np.float32)
    context = np.empty((B, 1, E), dtype=np.float32)
    for c in range(NCORES):
        sl = slice(c * NB, (c + 1) * NB)
        output[sl, 0, :] = res.results[c]["out"]
        context[sl, 0, :] = res.results[c]["ctx_out"]
    if _trace:
        return (output, context), res
    return (output, context)


if __name__ == "__main__":
    import reference

    inputs = {k: np.asarray(v) for k, v in reference.setup_inputs().items()}
    (o, c) = kernel(**inputs)
    print("output", o.shape, o.dtype, "context", c.shape, c.dtype)
